# revision 1
# baseline (speedup 1.0000x reference)
"""DampedLinOSSLayer Trainium2 kernel v3 (8 NeuronCores, batch-sharded).

Gauge factorization: x_t = e^{i*th*t} y_t turns the complex diagonal
recurrence into a real-coefficient scan y_t = r y_{t-1} + c_t.
Structure per core (4 batches):
  - input shipped fp16; transposed to [h, t] by the DMA xbar (no PE work)
  - B-proj per 512-chunk on PE (fp16, chunk phase folded into weights)
  - pre-rotation: 2 broadcast muls + 1 add per (chunk, half) on DVE fp16
    (2x mode); tables hold the in-chunk phase
  - 4 full-length (2048) scans per batch on DVE; fp32 coefficient and
    internal state, fp16 in/out
  - post-rotation muls on DVE; the re/im recombination adds are folded
    into 8 accumulating C matmuls (+1 for the D residual) per chunk
  - output written [h, t] fp32; host does the final transpose
"""

import numpy as np

BATCH, LENGTH, HIDDEN, P = 32, 2048, 128, 256
N_CORES = 8
BPC = BATCH // N_CORES
CH = 512
NCH = LENGTH // CH

XIN_DTYPE = np.float16

_COMPILED = {}


def _build_program():
    import concourse.bacc as bacc
    import concourse.mybir as mybir
    from concourse.tile import TileContext

    f32 = mybir.dt.float32
    fp16 = mybir.dt.float16
    mm = mybir.AluOpType.mult
    ad = mybir.AluOpType.add

    nc = bacc.Bacc("TRN2", target_bir_lowering=False, debug=False,
                   num_devices=N_CORES)

    xin = nc.dram_tensor("xin", [BPC, LENGTH, HIDDEN], fp16,
                         kind="ExternalInput").ap()
    # B weights, phase-folded per chunk: [h, J, comp, half, p]
    bw = nc.dram_tensor("bw", [HIDDEN, NCH, 2, 2, 128], fp16,
                        kind="ExternalInput").ap()
    # C weights per chunk: [p, J, half, wt(0=C're,1=-C'im), h]
    cw = nc.dram_tensor("cw", [128, NCH, 2, 2, HIDDEN], fp16,
                        kind="ExternalInput").ap()
    # in-chunk rotation tables: [p, half, tab, comp, CH]
    epre = nc.dram_tensor("epre", [128, 2, 2, 2, CH], fp16,
                          kind="ExternalInput").ap()
    epost = nc.dram_tensor("epost", [128, 2, 2, 2, CH], fp16,
                           kind="ExternalInput").ap()
    rcol = nc.dram_tensor("rcol", [128, 2], f32, kind="ExternalInput").ap()
    dw = nc.dram_tensor("dw", [HIDDEN, HIDDEN], fp16,
                        kind="ExternalInput").ap()
    # output in [h, t] layout; host transposes
    out = nc.dram_tensor("out", [BPC, HIDDEN, LENGTH], f32,
                         kind="ExternalOutput").ap()

    with TileContext(nc) as tc:
        with (
            tc.tile_pool(name="const", bufs=1) as cpool,
            tc.tile_pool(name="intp", bufs=2) as intr_pool,
            tc.tile_pool(name="bub", bufs=3) as bub_pool,
            tc.tile_pool(name="t12", bufs=3) as t12_pool,
            tc.tile_pool(name="cbuf", bufs=2) as cbuf_pool,
            tc.tile_pool(name="ybuf", bufs=2) as ybuf_pool,
            tc.tile_pool(name="xbuf", bufs=4) as xbuf_pool,
            tc.tile_pool(name="obuf", bufs=2) as obuf_pool,
            tc.tile_pool(name="psb", bufs=3, space="PSUM") as psb,
            tc.tile_pool(name="pso", bufs=2, space="PSUM") as pso,
        ):
            bw_t = cpool.tile([HIDDEN, NCH, 2, 2, 128], fp16, tag="bw")
            cw_t = cpool.tile([128, NCH, 2, 2, HIDDEN], fp16, tag="cw")
            epre_t = cpool.tile([128, 2, 2, 2, CH], fp16, tag="epre")
            epost_t = cpool.tile([128, 2, 2, 2, CH], fp16, tag="epost")
            rcol_t = cpool.tile([128, 2], f32, tag="rcol")
            dw_t = cpool.tile([HIDDEN, HIDDEN], fp16, tag="dw")
            for src, dst in [(bw, bw_t), (epre, epre_t), (rcol, rcol_t)]:
                nc.sync.dma_start(dst[:], src[:])
            # batch-0 input transpose issued before the constants that are
            # only needed later (cw/epost/dw) so the first B-matmul and
            # pre-rotation start ~4us earlier.
            inT0 = intr_pool.tile([HIDDEN, LENGTH], fp16, tag="inT")
            nc.sync.dma_start_transpose(inT0[:], xin[0])
            for src, dst in [(cw, cw_t), (epost, epost_t), (dw, dw_t)]:
                nc.sync.dma_start(dst[:], src[:])

            # scan coefficient, f32, broadcast along t: [128, half, LENGTH]
            rbc = cpool.tile([128, 2, LENGTH], f32, tag="rbc")
            for half in range(2):
                nc.vector.memset(rbc[:, half], 1.0)
                nc.vector.tensor_scalar_mul(
                    rbc[:, half], rbc[:, half], rcol_t[:, half:half + 1])

            for b in range(BPC):
                # ---- input transpose via DMA xbar: inT [h, t] fp16 ----
                if b == 0:
                    inT = inT0
                else:
                    inT = intr_pool.tile([HIDDEN, LENGTH], fp16, tag="inT")
                    nc.sync.dma_start_transpose(inT[:], xin[b])

                # ---- B-proj + pre-rotation into cf streams ----
                cf = [cbuf_pool.tile([128, 2, LENGTH], fp16, tag=f"cf{h}",
                                     name=f"cf{h}")
                      for h in range(2)]
                for J in range(NCH):
                    tsl = slice(CH * J, CH * (J + 1))
                    for half in range(2):
                        bu = psb.tile([128, 2, CH], f32, tag="bu")
                        for comp in range(2):
                            nc.tensor.matmul(
                                bu[:, comp, :], bw_t[:, J, comp, half],
                                inT[:, tsl], start=True, stop=True)
                        bub = bub_pool.tile([128, 2, CH], fp16, tag="bub")
                        nc.scalar.copy(bub[:], bu[:])
                        # A = bur (bcast) * [cos | -sin] ; B = bui * [sin | cos]
                        # cf[cre|cim] = A + B, all operands contiguous
                        t12 = t12_pool.tile([128, 2, 2, CH], fp16, tag="t12")
                        nc.vector.tensor_mul(
                            t12[:, 0],
                            bub[:, 0:1, :].to_broadcast([128, 2, CH]),
                            epre_t[:, half, 0])
                        nc.vector.tensor_mul(
                            t12[:, 1],
                            bub[:, 1:2, :].to_broadcast([128, 2, CH]),
                            epre_t[:, half, 1])
                        nc.vector.tensor_add(cf[half][:, :, tsl], t12[:, 0],
                                             t12[:, 1])

                # ---- scans: y[half][comp, :] over full length ----
                yt = [ybuf_pool.tile([128, 2, LENGTH], fp16, tag=f"y{h}",
                                     name=f"y{h}")
                      for h in range(2)]
                for half in range(2):
                    for comp in range(2):
                        nc.vector.tensor_tensor_scan(
                            yt[half][:, comp, :], rbc[:, half],
                            cf[half][:, comp, :], 0.0, op0=mm, op1=ad)

                # ---- post-rotation + C-proj + D ----
                for J in range(NCH):
                    tsl = slice(CH * J, CH * (J + 1))
                    outT = pso.tile([HIDDEN, CH], f32, tag="outT")
                    first = True
                    for half in range(2):
                        # one fused mul: tab0 planes (y*[cos|-sin]) feed the
                        # C're matmuls, tab1 planes (y*[sin|cos]) feed -C'im
                        t34 = xbuf_pool.tile([128, 2, 2, CH], fp16,
                                             tag="t34")
                        nc.vector.tensor_mul(
                            t34[:],
                            yt[half][:, :, tsl]
                            .rearrange("p (o c) t -> p o c t", o=1)
                            .to_broadcast([128, 2, 2, CH]),
                            epost_t[:, half])
                        for wt in range(2):
                            for comp in range(2):
                                nc.tensor.matmul(
                                    outT[:], cw_t[:, J, half, wt],
                                    t34[:, wt, comp, :],
                                    start=first, stop=False)
                                first = False
                    nc.tensor.matmul(outT[:], dw_t[:], inT[:, tsl],
                                     start=False, stop=True)
                    oT = obuf_pool.tile([HIDDEN, CH], f32, tag="oT")
                    nc.scalar.copy(oT[:], outT[:])
                    nc.sync.dma_start(out[b, :, tsl], oT[:])

    nc.compile()
    return nc


def _host_constants(A_diag, G_diag, steps, B, C, D):
    A = A_diag.astype(np.float64)
    G = G_diag.astype(np.float64)
    st = steps.astype(np.float64)
    step = 1.0 / (1.0 + np.exp(-st))
    g = np.maximum(G, 0.0)
    denom = np.maximum(step * step, 1e-6)
    s = step * g
    base = np.sqrt(np.maximum(1.0 + s, 1e-6))
    a_low = (2.0 + s - 2.0 * base) / denom
    a_high = (2.0 + s + 2.0 * base) / denom
    a = a_low + np.maximum(A - a_low, 0.0) - np.maximum(A - a_high, 0.0)
    S = 1.0 / (1.0 + step * g)
    T = S + 1.0 - step * step * S * a
    imag = np.sqrt(np.maximum(S - 0.25 * T * T, 0.0))
    lam = 0.5 * T + 1j * imag
    r = np.abs(lam)
    th = np.angle(lam)

    j0 = np.arange(CH, dtype=np.float64)
    cos_m = np.cos(th[:, None] * j0[None, :])
    sin_m = np.sin(th[:, None] * j0[None, :])

    # epre: tab0 applied to broadcast(bur): [cos | -sin]
    #       tab1 applied to broadcast(bui): [sin | cos]
    #   so cf = A + B gives cre = bur*cos + bui*sin ;
    #                       cim = -bur*sin + bui*cos
    # epost planes for the C-matmul fold:
    #   tab0 (weights C're): [cos | -sin] ; tab1 (weights -C'im): [sin | cos]
    epre = np.zeros((128, 2, 2, 2, CH), np.float16)
    epost = np.zeros((128, 2, 2, 2, CH), np.float16)
    for half in range(2):
        psl = slice(128 * half, 128 * (half + 1))
        epre[:, half, 0, 0] = cos_m[psl]
        epre[:, half, 0, 1] = -sin_m[psl]
        epre[:, half, 1, 0] = sin_m[psl]
        epre[:, half, 1, 1] = cos_m[psl]
        epost[:, half, 0, 0] = cos_m[psl]
        epost[:, half, 0, 1] = -sin_m[psl]
        epost[:, half, 1, 0] = sin_m[psl]
        epost[:, half, 1, 1] = cos_m[psl]

    Bc = B[..., 0].astype(np.float64) + 1j * B[..., 1].astype(np.float64)
    Cc = C[..., 0].astype(np.float64) + 1j * C[..., 1].astype(np.float64)
    bw = np.zeros((HIDDEN, NCH, 2, 2, 128), np.float16)
    cwt = np.zeros((128, NCH, 2, 2, HIDDEN), np.float16)
    for J in range(NCH):
        ph = np.exp(-1j * th * (CH * J))
        BJ = Bc * ph[:, None]
        phc = np.exp(+1j * th * (CH * J))
        CT = Cc * phc[None, :]                     # [H, P]
        for half in range(2):
            psl = slice(128 * half, 128 * (half + 1))
            bw[:, J, 0, half] = BJ.real[psl].T
            bw[:, J, 1, half] = BJ.imag[psl].T
            # wt 0: C're ; wt 1: -C'im   (lhsT [p, h])
            cwt[:, J, half, 0] = CT.real[:, psl].T
            cwt[:, J, half, 1] = -CT.imag[:, psl].T

    rcol = np.zeros((128, 2), np.float32)
    rcol[:, 0] = r[:128]
    rcol[:, 1] = r[128:]
    dwm = np.diag(D.astype(np.float64)).astype(np.float16)
    return dict(bw=bw, cw=cwt, epre=epre, epost=epost, rcol=rcol, dw=dwm)


def kernel(inputs, A_diag, G_diag, steps, B, C, D):
    from concourse import bass_utils

    inputs = np.asarray(inputs, np.float32)
    consts = _host_constants(np.asarray(A_diag), np.asarray(G_diag),
                             np.asarray(steps), np.asarray(B), np.asarray(C),
                             np.asarray(D))

    if "prog" not in _COMPILED:
        _COMPILED["prog"] = _build_program()
    nc = _COMPILED["prog"]

    in_maps = []
    for core in range(N_CORES):
        m = dict(consts)
        m["xin"] = np.ascontiguousarray(
            inputs[BPC * core: BPC * (core + 1)]).astype(np.float16)
        in_maps.append(m)
    res = bass_utils.run_bass_kernel_spmd(nc, in_maps,
                                          core_ids=list(range(N_CORES)))
    out = np.concatenate([res.results[i]["out"] for i in range(N_CORES)],
                         axis=0)                      # [B, H, L]
    return np.ascontiguousarray(out.transpose(0, 2, 1)).astype(np.float32)



# revision 3
# speedup vs baseline: 2.4137x; 2.4137x over previous
"""DampedLinOSSLayer Trainium2 kernel v4 (8 NeuronCores, batch-sharded).

Radix-4 time decimation on top of the gauge-factorized scan:
  x_t = lam x_{t-1} + B u_t  decimated by Q=4: z_k = lam^4 z_{k-1} + d_k with
  d_k = sum_j lam^{3-j} B u_{4k+j}.  The lam^{3-j} factors fold into 4 copies
  of the B weights, so the intra-group reduction runs on the PE (4
  accumulating matmuls over phase-deinterleaved input columns).  The DVE only
  rotates/scans the decimated (L/4) grid: gauge y_k = r^4 y_{k-1} + cf_k with
  cf = e^{-i4th k} d.  Output for t = 4k+j:
    out = Re(C lam^{j+1} z_{k-1}) + sum_{i<=j} M_{j-i} u_{4k+i}
  with M_d = Re(C lam^d B) + diag(D)[d=0] -- the z-term uses 4 per-phase C
  weight copies against rotated y-planes (shifted one column via the matmul
  output AP); the u-term is a 10-matmul causal phase convolution on the PE.
  DVE work (rotations + scan) drops 4x vs the undecimated kernel; the PE
  absorbs it as dense fp16 matmuls.

Host side: input is pre-transposed/deinterleaved to [H, 4, 512] per batch
(plain contiguous DMA, no device transpose); output phases [4, H, 512] fp16
are re-interleaved on the host.
"""

import numpy as np

BATCH, LENGTH, HIDDEN, P = 32, 2048, 128, 256
N_CORES = 8
BPC = BATCH // N_CORES
Q = 4
K = LENGTH // Q

XIN_DTYPE = np.float16

_COMPILED = {}


def _build_program():
    import concourse.bacc as bacc
    import concourse.mybir as mybir
    from concourse.tile import TileContext

    f32 = mybir.dt.float32
    fp16 = mybir.dt.float16
    mm = mybir.AluOpType.mult
    ad = mybir.AluOpType.add

    nc = bacc.Bacc("TRN2", target_bir_lowering=False, debug=False,
                   num_devices=N_CORES)

    # host-deinterleaved input: xin[b, h, j, k] = u[b, 4k+j, h]
    xin = nc.dram_tensor("xin", [BPC, HIDDEN, Q, K], fp16,
                         kind="ExternalInput").ap()
    # B weights with lam^{3-j} folded: [h, half, comp, j, p]
    bw = nc.dram_tensor("bw", [HIDDEN, 2, 2, Q, 128], fp16,
                        kind="ExternalInput").ap()
    # per-phase C weights C*lam^{j+1}: [p, half, j, wt(0=re,1=-im), h]
    cw = nc.dram_tensor("cw", [128, 2, Q, 2, HIDDEN], fp16,
                        kind="ExternalInput").ap()
    # phase-convolution weights M_d^T: [h_in, d, h_out]
    m2w = nc.dram_tensor("m2w", [HIDDEN, Q, HIDDEN], fp16,
                         kind="ExternalInput").ap()
    # rotation tables on the decimated grid: [p, half, tab, plane, k]
    epre = nc.dram_tensor("epre", [128, 2, 2, 2, K], fp16,
                          kind="ExternalInput").ap()
    epost = nc.dram_tensor("epost", [128, 2, 2, 2, K], fp16,
                           kind="ExternalInput").ap()
    # merged-scan coefficient r^4 with zeros at segment starts: [p, 4*K]
    rz = nc.dram_tensor("rz", [128, 4 * K], f32, kind="ExternalInput").ap()
    # output phases; host interleaves
    out = nc.dram_tensor("out", [BPC, Q, HIDDEN, K], fp16,
                         kind="ExternalOutput").ap()

    with TileContext(nc) as tc:
        with (
            tc.tile_pool(name="const", bufs=1) as cpool,
            tc.tile_pool(name="intp", bufs=3) as intr_pool,
            tc.tile_pool(name="dsb", bufs=2) as dsb_pool,
            tc.tile_pool(name="t12", bufs=2) as t12_pool,
            tc.tile_pool(name="cbuf", bufs=2) as cbuf_pool,
            tc.tile_pool(name="ybuf", bufs=2) as ybuf_pool,
            tc.tile_pool(name="xbuf", bufs=4) as xbuf_pool,
            tc.tile_pool(name="obuf", bufs=8) as obuf_pool,
            tc.tile_pool(name="psb", bufs=2, space="PSUM") as psb,
            tc.tile_pool(name="pso", bufs=4, space="PSUM") as pso,
        ):
            bw_t = cpool.tile([HIDDEN, 2, 2, Q, 128], fp16, tag="bw")
            cw_t = cpool.tile([128, 2, Q, 2, HIDDEN], fp16, tag="cw")
            m2w_t = cpool.tile([HIDDEN, Q, HIDDEN], fp16, tag="m2w")
            epre_t = cpool.tile([128, 2, 2, 2, K], fp16, tag="epre")
            epost_t = cpool.tile([128, 2, 2, 2, K], fp16, tag="epost")
            rz_t = cpool.tile([128, 4 * K], f32, tag="rz")
            # constants needed first go first; input b=0 interleaved between
            for src, dst in [(bw, bw_t), (epre, epre_t), (rz, rz_t)]:
                nc.sync.dma_start(dst[:], src[:])
            inT = [None] * (BPC + 1)
            inT[0] = intr_pool.tile([HIDDEN, Q, K], fp16, tag="inT", name="inT0")
            nc.sync.dma_start(inT[0][:], xin[0])
            for src, dst in [(cw, cw_t), (epost, epost_t), (m2w, m2w_t)]:
                nc.sync.dma_start(dst[:], src[:])

            dsb = [None] * (BPC + 1)

            def bproj(b):
                # B-projection of batch b: 4 accumulating matmuls per
                # (half, comp) over the phase-deinterleaved input columns,
                # then PSUM->SBUF fp16 copy per half.
                dsb[b] = [None, None]
                for half in range(2):
                    d = psb.tile([128, 2, K], f32, tag="d")
                    for comp in range(2):
                        for j in range(Q):
                            nc.tensor.matmul(
                                d[:, comp], bw_t[:, half, comp, j],
                                inT[b][:, j], start=(j == 0), stop=(j == Q - 1))
                    ds = dsb_pool.tile([128, 2, K], fp16, tag="ds",
                                       name=f"ds{b}_{half}")
                    nc.scalar.copy(ds[:], d[:])
                    dsb[b][half] = ds

            bproj(0)
            for b in range(BPC):
                # prefetch + B-proj one batch ahead so the DVE chain of
                # batch b never waits on the PE/ScalarE front end.
                if b + 1 < BPC:
                    inT[b + 1] = intr_pool.tile([HIDDEN, Q, K], fp16,
                                                tag="inT",
                                                name=f"inT{b + 1}")
                    nc.sync.dma_start(inT[b + 1][:], xin[b + 1])
                    bproj(b + 1)

                # ---- pre-rotation into cf (gauge frame scan input) ----
                cf = cbuf_pool.tile([128, 2, 2, K], fp16, tag="cf",
                                    name=f"cf{b}")
                for half in range(2):
                    t12 = t12_pool.tile([128, 2, 2, K], fp16, tag="t12")
                    nc.vector.tensor_mul(
                        t12[:, 0],
                        dsb[b][half][:, 0:1, :].to_broadcast([128, 2, K]),
                        epre_t[:, half, 0])
                    nc.vector.tensor_mul(
                        t12[:, 1],
                        dsb[b][half][:, 1:2, :].to_broadcast([128, 2, K]),
                        epre_t[:, half, 1])
                    nc.vector.tensor_add(cf[:, half], t12[:, 0], t12[:, 1])
                dsb[b] = None

                # ---- one merged scan over (half, comp) segments ----
                yb = ybuf_pool.tile([128, 2, 2, K], fp16, tag="y",
                                    name=f"y{b}")
                nc.vector.tensor_tensor_scan(
                    yb[:].rearrange("p a c k -> p (a c k)"), rz_t[:],
                    cf[:].rearrange("p a c k -> p (a c k)"), 0.0,
                    op0=mm, op1=ad)

                # ---- post-rotation planes for the C projection ----
                t34 = [None, None]
                for half in range(2):
                    t34[half] = xbuf_pool.tile([128, 2, 2, K], fp16,
                                               tag="t34",
                                               name=f"t34_{b}_{half}")
                    nc.vector.tensor_mul(
                        t34[half][:],
                        yb[:, half]
                        .rearrange("p (o c) k -> p o c k", o=1)
                        .to_broadcast([128, 2, 2, K]),
                        epost_t[:, half])

                # ---- per-phase outputs: u-convolution + shifted z-term ----
                for j in range(Q):
                    oj = pso.tile([HIDDEN, K], f32, tag="oj")
                    for i in range(j + 1):
                        nc.tensor.matmul(oj[:], m2w_t[:, j - i], inT[b][:, i],
                                         start=(i == 0), stop=False)
                    for half in range(2):
                        for wt in range(2):
                            for comp in range(2):
                                last = (half == 1 and wt == 1 and comp == 1)
                                nc.tensor.matmul(
                                    oj[:, 1:K], cw_t[:, half, j, wt],
                                    t34[half][:, wt, comp, 0:K - 1],
                                    start=False, stop=last)
                    oT = obuf_pool.tile([HIDDEN, K], fp16, tag="oT")
                    nc.scalar.copy(oT[:], oj[:])
                    nc.sync.dma_start(out[b, j], oT[:])

    nc.compile()
    return nc


def _host_constants(A_diag, G_diag, steps, B, C, D):
    A = A_diag.astype(np.float64)
    G = G_diag.astype(np.float64)
    st = steps.astype(np.float64)
    step = 1.0 / (1.0 + np.exp(-st))
    g = np.maximum(G, 0.0)
    denom = np.maximum(step * step, 1e-6)
    s = step * g
    base = np.sqrt(np.maximum(1.0 + s, 1e-6))
    a_low = (2.0 + s - 2.0 * base) / denom
    a_high = (2.0 + s + 2.0 * base) / denom
    a = a_low + np.maximum(A - a_low, 0.0) - np.maximum(A - a_high, 0.0)
    S = 1.0 / (1.0 + step * g)
    T = S + 1.0 - step * step * S * a
    imag = np.sqrt(np.maximum(S - 0.25 * T * T, 0.0))
    lam = 0.5 * T + 1j * imag
    r = np.abs(lam)
    th = np.angle(lam)

    Bc = B[..., 0].astype(np.float64) + 1j * B[..., 1].astype(np.float64)
    Cc = C[..., 0].astype(np.float64) + 1j * C[..., 1].astype(np.float64)

    # bw[h, half, comp, j, p] = {Re,Im}(lam^{3-j} Bc)[p, h]
    bw = np.zeros((HIDDEN, 2, 2, Q, 128), np.float16)
    # cw[p, half, j, wt, h]: wt0 = Re(C lam^{j+1})^T, wt1 = -Im(C lam^{j+1})^T
    cw = np.zeros((128, 2, Q, 2, HIDDEN), np.float16)
    for j in range(Q):
        Wj = (lam ** (Q - 1 - j))[:, None] * Bc          # [P, H]
        Cj = Cc * (lam ** (j + 1))[None, :]              # [H, P]
        for half in range(2):
            psl = slice(128 * half, 128 * (half + 1))
            bw[:, half, 0, j] = Wj.real[psl].T
            bw[:, half, 1, j] = Wj.imag[psl].T
            cw[:, half, j, 0] = Cj.real[:, psl].T
            cw[:, half, j, 1] = -Cj.imag[:, psl].T

    # m2w[h_in, d, h_out] = M_d^T with M_d = Re(C lam^d B) (+diag(D) at d=0)
    m2w = np.zeros((HIDDEN, Q, HIDDEN), np.float16)
    for d in range(Q):
        Md = np.real(Cc @ ((lam ** d)[:, None] * Bc))
        if d == 0:
            Md = Md + np.diag(D.astype(np.float64))
        m2w[:, d] = Md.T

    # rotation tables on the decimated grid, phase phi = 4*th*k
    kk = np.arange(K, dtype=np.float64)
    cos_m = np.cos(Q * th[:, None] * kk[None, :])
    sin_m = np.sin(Q * th[:, None] * kk[None, :])
    epre = np.zeros((128, 2, 2, 2, K), np.float16)
    epost = np.zeros((128, 2, 2, 2, K), np.float16)
    for half in range(2):
        psl = slice(128 * half, 128 * (half + 1))
        # pre: tab0 * bcast(d_re) = [cos | -sin]; tab1 * bcast(d_im) = [sin | cos]
        epre[:, half, 0, 0] = cos_m[psl]
        epre[:, half, 0, 1] = -sin_m[psl]
        epre[:, half, 1, 0] = sin_m[psl]
        epre[:, half, 1, 1] = cos_m[psl]
        # post: wt0 (weights Re C_j): [cos | -sin]; wt1 (-Im C_j): [sin | cos]
        epost[:, half, 0, 0] = cos_m[psl]
        epost[:, half, 0, 1] = -sin_m[psl]
        epost[:, half, 1, 0] = sin_m[psl]
        epost[:, half, 1, 1] = cos_m[psl]

    # merged-scan coefficient: r^4 per (p, half), zero at each segment start
    r4 = (r ** Q).astype(np.float64)
    rz = np.zeros((128, 2, 2, K), np.float32)
    for half in range(2):
        psl = slice(128 * half, 128 * (half + 1))
        rz[:, half, :, :] = r4[psl][:, None, None]
    rz[:, :, :, 0] = 0.0
    rz = rz.reshape(128, 4 * K)

    return dict(bw=bw, cw=cw, m2w=m2w, epre=epre, epost=epost, rz=rz)


def _prep_xin(core_inputs_f32):
    """[BPC, L, H] f32 -> [BPC, H, Q, K] fp16 with xin[b,h,j,k] = u[b,4k+j,h]."""
    a = core_inputs_f32.reshape(BPC, K, Q, HIDDEN)
    return np.ascontiguousarray(a.transpose(0, 3, 2, 1)).astype(np.float16)


def _make_in_maps(inputs, A_diag, G_diag, steps, B, C, D):
    inputs = np.asarray(inputs, np.float32)
    consts = _host_constants(np.asarray(A_diag), np.asarray(G_diag),
                             np.asarray(steps), np.asarray(B), np.asarray(C),
                             np.asarray(D))
    in_maps = []
    for core in range(N_CORES):
        m = dict(consts)
        m["xin"] = _prep_xin(inputs[BPC * core: BPC * (core + 1)])
        in_maps.append(m)
    return in_maps


def kernel(inputs, A_diag, G_diag, steps, B, C, D):
    from concourse import bass_utils

    in_maps = _make_in_maps(inputs, A_diag, G_diag, steps, B, C, D)
    if "prog" not in _COMPILED:
        _COMPILED["prog"] = _build_program()
    nc = _COMPILED["prog"]

    res = bass_utils.run_bass_kernel_spmd(nc, in_maps,
                                          core_ids=list(range(N_CORES)))
    out = np.concatenate([res.results[i]["out"] for i in range(N_CORES)],
                         axis=0)                      # [B, Q, H, K] fp16
    # out[b, j, h, k] -> full[b, 4k+j, h]
    full = out.astype(np.float32).transpose(0, 3, 1, 2)   # [B, K, Q, H]
    return np.ascontiguousarray(full.reshape(BATCH, LENGTH, HIDDEN))


# revision 4
# speedup vs baseline: 2.4613x; 1.0197x over previous
"""DampedLinOSSLayer Trainium2 kernel v5 (8 NeuronCores, batch-sharded).

Radix-4 time decimation on top of the gauge-factorized scan:
  x_t = lam x_{t-1} + B u_t  decimated by Q=4: z_k = lam^4 z_{k-1} + d_k with
  d_k = sum_j lam^{3-j} B u_{4k+j}.  The lam^{3-j} factors fold into 4 copies
  of the B weights, so the intra-group reduction runs on the PE (4
  accumulating matmuls over phase-deinterleaved input columns).  The DVE only
  rotates/scans the decimated (L/4) grid: gauge y_k = r^4 y_{k-1} + cf_k with
  cf = e^{-i4th k} d; one merged scan instruction covers all four
  (half, comp) segments via zeroed coefficient columns at segment starts.
  Output for t = 4k+j:
    out = Re(C lam^{j+1} z_{k-1}) + sum_{i<=j} M_{j-i} u_{4k+i}
  with M_d = Re(C lam^d B) + diag(D)[d=0].  The z-planes are combined on the
  DVE (z_re, z_im) so the C projection is 2 matmuls per (half, phase); the
  one-column shift of z rides the matmul output AP.  The u-term is a
  10-matmul causal phase convolution.  All rotation tables are plane-views
  of a single e^{+-i4th k} table.

Host side: input is pre-transposed/deinterleaved to [H, 4, 512] per batch
(plain contiguous DMA, no device transpose); output phases [4, H, 512] fp16
are re-interleaved on the host.
"""

import numpy as np

BATCH, LENGTH, HIDDEN, P = 32, 2048, 128, 256
N_CORES = 8
BPC = BATCH // N_CORES
Q = 4
K = LENGTH // Q

XIN_DTYPE = np.float16

_COMPILED = {}


def _build_program():
    import concourse.bacc as bacc
    import concourse.mybir as mybir
    from concourse.tile import TileContext

    f32 = mybir.dt.float32
    fp16 = mybir.dt.float16
    mm = mybir.AluOpType.mult
    ad = mybir.AluOpType.add

    nc = bacc.Bacc("TRN2", target_bir_lowering=False, debug=False,
                   num_devices=N_CORES)

    # host-deinterleaved input: xin[b, h, j, k] = u[b, 4k+j, h]
    xin = nc.dram_tensor("xin", [BPC, HIDDEN, Q, K], fp16,
                         kind="ExternalInput").ap()
    # B weights with lam^{3-j} folded: [h, half, comp, j, p]
    bw = nc.dram_tensor("bw", [HIDDEN, 2, 2, Q, 128], fp16,
                        kind="ExternalInput").ap()
    # per-phase C weights C*lam^{j+1}: [p, half, j, wt(0=re,1=-im), h]
    cw = nc.dram_tensor("cw", [128, 2, Q, 2, HIDDEN], fp16,
                        kind="ExternalInput").ap()
    # phase-convolution weights M_d^T: [h_in, d, h_out]
    m2w = nc.dram_tensor("m2w", [HIDDEN, Q, HIDDEN], fp16,
                         kind="ExternalInput").ap()
    # rotation tables, phase 4*th*k: [p, half, tab, plane, k]
    #   tab0 = [cos | -sin], tab1 = [sin | cos]
    epre = nc.dram_tensor("epre", [128, 2, 2, 2, K], fp16,
                          kind="ExternalInput").ap()
    # r^4 per (p, half) for the scan coefficient
    rcol = nc.dram_tensor("rcol", [128, 2], f32, kind="ExternalInput").ap()
    # output phases; host interleaves
    out = nc.dram_tensor("out", [BPC, Q, HIDDEN, K], fp16,
                         kind="ExternalOutput").ap()

    with TileContext(nc) as tc:
        with (
            tc.tile_pool(name="const", bufs=1) as cpool,
            tc.tile_pool(name="intp", bufs=3) as intr_pool,
            tc.tile_pool(name="dsb", bufs=2) as dsb_pool,
            tc.tile_pool(name="t12", bufs=2) as t12_pool,
            tc.tile_pool(name="cbuf", bufs=2) as cbuf_pool,
            tc.tile_pool(name="ybuf", bufs=2) as ybuf_pool,
            tc.tile_pool(name="xbuf", bufs=2) as xbuf_pool,
            tc.tile_pool(name="obuf", bufs=8) as obuf_pool,
            tc.tile_pool(name="psb", bufs=2, space="PSUM") as psb,
            tc.tile_pool(name="pso", bufs=4, space="PSUM") as pso,
        ):
            bw_t = cpool.tile([HIDDEN, 2, 2, Q, 128], fp16, tag="bw")
            cw_t = cpool.tile([128, 2, Q, 2, HIDDEN], fp16, tag="cw")
            m2w_t = cpool.tile([HIDDEN, Q, HIDDEN], fp16, tag="m2w")
            epre_t = cpool.tile([128, 2, 2, 2, K], fp16, tag="epre")
            rcol_t = cpool.tile([128, 2], f32, tag="rcol")
            # DMA order = need order: batch-0 B-proj wants bw+xin first, the
            # first pre-rotation wants epre+rcol, the C projection cw/m2w.
            inT = [None] * (BPC + 1)
            inT[0] = intr_pool.tile([HIDDEN, Q, K], fp16, tag="inT",
                                    name="inT0")
            for src, dst in [(bw, bw_t), (xin[0], inT[0]), (epre, epre_t),
                             (rcol, rcol_t), (cw, cw_t), (m2w, m2w_t)]:
                nc.sync.dma_start(dst[:], src[:])

            # scan coefficient [128, (half comp k)]: r^4, zero at segment
            # starts so one scan instruction covers 4 independent segments.
            rz_t = cpool.tile([128, 2, 2, K], f32, tag="rz")
            for half in range(2):
                nc.vector.memset(rz_t[:, half], 1.0)
                nc.vector.tensor_scalar_mul(
                    rz_t[:, half], rz_t[:, half],
                    rcol_t[:, half:half + 1])
            nc.vector.memset(rz_t[:, :, :, 0:1], 0.0)

            dsb = [None] * (BPC + 1)

            def bproj(b):
                # B-projection of batch b: 4 accumulating matmuls per
                # (half, comp) over the phase-deinterleaved input columns,
                # then PSUM->SBUF fp16 copies into one [p, half, comp, k]
                # tile.
                ds = dsb_pool.tile([128, 2, 2, K], fp16, tag="ds",
                                   name=f"ds{b}")
                for half in range(2):
                    d = psb.tile([128, 2, K], f32, tag="d")
                    for comp in range(2):
                        for j in range(Q):
                            nc.tensor.matmul(
                                d[:, comp], bw_t[:, half, comp, j],
                                inT[b][:, j], start=(j == 0), stop=(j == Q - 1))
                    nc.scalar.copy(ds[:, half], d[:])
                dsb[b] = ds

            bproj(0)
            for b in range(BPC):
                # prefetch + B-proj one batch ahead so the DVE chain of
                # batch b never waits on the PE/ScalarE front end.
                if b + 1 < BPC:
                    inT[b + 1] = intr_pool.tile([HIDDEN, Q, K], fp16,
                                                tag="inT",
                                                name=f"inT{b + 1}")
                    nc.sync.dma_start(inT[b + 1][:], xin[b + 1])
                    bproj(b + 1)

                # ---- pre-rotation into cf (gauge-frame scan input) ----
                # cf_re = d_re cos + d_im sin ; cf_im = -d_re sin + d_im cos
                cf = cbuf_pool.tile([128, 2, 2, K], fp16, tag="cf",
                                    name=f"cf{b}")
                t12 = t12_pool.tile([128, 2, 2, 2, K], fp16, tag="t12")
                nc.vector.tensor_mul(
                    t12[:, 0],
                    dsb[b][:, :, 0:1, :].to_broadcast([128, 2, 2, K]),
                    epre_t[:, :, 0])
                nc.vector.tensor_mul(
                    t12[:, 1],
                    dsb[b][:, :, 1:2, :].to_broadcast([128, 2, 2, K]),
                    epre_t[:, :, 1])
                nc.vector.tensor_add(cf[:], t12[:, 0], t12[:, 1])
                dsb[b] = None

                # ---- one merged scan over the (half, comp) segments ----
                yb = ybuf_pool.tile([128, 2, 2, K], fp16, tag="y",
                                    name=f"y{b}")
                nc.vector.tensor_tensor_scan(
                    yb[:].rearrange("p a c k -> p (a c k)"),
                    rz_t[:].rearrange("p a c k -> p (a c k)"),
                    cf[:].rearrange("p a c k -> p (a c k)"), 0.0,
                    op0=mm, op1=ad)

                # ---- post-rotation, combined into (z_re, z_im) planes ----
                # z_re = y_re cos - y_im sin ; z_im = y_re sin + y_im cos
                # tables are plane-views of epre: [cos|sin] and [-sin|cos]
                zt = xbuf_pool.tile([128, 2, 2, K], fp16, tag="zt",
                                    name=f"zt{b}")
                t34 = t12_pool.tile([128, 2, 2, 2, K], fp16, tag="t12",
                                    name=f"t34_{b}")
                nc.vector.tensor_mul(
                    t34[:, 0],
                    yb[:, :, 0:1, :].to_broadcast([128, 2, 2, K]),
                    epre_t[:, :, :, 0, :])
                nc.vector.tensor_mul(
                    t34[:, 1],
                    yb[:, :, 1:2, :].to_broadcast([128, 2, 2, K]),
                    epre_t[:, :, :, 1, :])
                nc.vector.tensor_add(zt[:], t34[:, 0], t34[:, 1])

                # ---- per-phase outputs: u-convolution + shifted z-term ----
                for j in range(Q):
                    oj = pso.tile([HIDDEN, K], f32, tag="oj")
                    for i in range(j + 1):
                        nc.tensor.matmul(oj[:], m2w_t[:, j - i], inT[b][:, i],
                                         start=(i == 0), stop=False)
                    for half in range(2):
                        for wt in range(2):
                            last = (half == 1 and wt == 1)
                            nc.tensor.matmul(
                                oj[:, 1:K], cw_t[:, half, j, wt],
                                zt[:, half, wt, 0:K - 1],
                                start=False, stop=last)
                    oT = obuf_pool.tile([HIDDEN, K], fp16, tag="oT")
                    nc.scalar.copy(oT[:], oj[:])
                    nc.sync.dma_start(out[b, j], oT[:])

    nc.compile()
    return nc


def _host_constants(A_diag, G_diag, steps, B, C, D):
    A = A_diag.astype(np.float64)
    G = G_diag.astype(np.float64)
    st = steps.astype(np.float64)
    step = 1.0 / (1.0 + np.exp(-st))
    g = np.maximum(G, 0.0)
    denom = np.maximum(step * step, 1e-6)
    s = step * g
    base = np.sqrt(np.maximum(1.0 + s, 1e-6))
    a_low = (2.0 + s - 2.0 * base) / denom
    a_high = (2.0 + s + 2.0 * base) / denom
    a = a_low + np.maximum(A - a_low, 0.0) - np.maximum(A - a_high, 0.0)
    S = 1.0 / (1.0 + step * g)
    T = S + 1.0 - step * step * S * a
    imag = np.sqrt(np.maximum(S - 0.25 * T * T, 0.0))
    lam = 0.5 * T + 1j * imag
    r = np.abs(lam)
    th = np.angle(lam)

    Bc = B[..., 0].astype(np.float64) + 1j * B[..., 1].astype(np.float64)
    Cc = C[..., 0].astype(np.float64) + 1j * C[..., 1].astype(np.float64)

    # bw[h, half, comp, j, p] = {Re,Im}(lam^{3-j} Bc)[p, h]
    bw = np.zeros((HIDDEN, 2, 2, Q, 128), np.float16)
    # cw[p, half, j, wt, h]: wt0 = Re(C lam^{j+1})^T, wt1 = -Im(C lam^{j+1})^T
    cw = np.zeros((128, 2, Q, 2, HIDDEN), np.float16)
    for j in range(Q):
        Wj = (lam ** (Q - 1 - j))[:, None] * Bc          # [P, H]
        Cj = Cc * (lam ** (j + 1))[None, :]              # [H, P]
        for half in range(2):
            psl = slice(128 * half, 128 * (half + 1))
            bw[:, half, 0, j] = Wj.real[psl].T
            bw[:, half, 1, j] = Wj.imag[psl].T
            cw[:, half, j, 0] = Cj.real[:, psl].T
            cw[:, half, j, 1] = -Cj.imag[:, psl].T

    # m2w[h_in, d, h_out] = M_d^T with M_d = Re(C lam^d B) (+diag(D) at d=0)
    m2w = np.zeros((HIDDEN, Q, HIDDEN), np.float16)
    for d in range(Q):
        Md = np.real(Cc @ ((lam ** d)[:, None] * Bc))
        if d == 0:
            Md = Md + np.diag(D.astype(np.float64))
        m2w[:, d] = Md.T

    # rotation tables, phase phi = 4*th*k: tab0 = [cos|-sin], tab1 = [sin|cos]
    kk = np.arange(K, dtype=np.float64)
    cos_m = np.cos(Q * th[:, None] * kk[None, :])
    sin_m = np.sin(Q * th[:, None] * kk[None, :])
    epre = np.zeros((128, 2, 2, 2, K), np.float16)
    for half in range(2):
        psl = slice(128 * half, 128 * (half + 1))
        epre[:, half, 0, 0] = cos_m[psl]
        epre[:, half, 0, 1] = -sin_m[psl]
        epre[:, half, 1, 0] = sin_m[psl]
        epre[:, half, 1, 1] = cos_m[psl]

    rcol = np.zeros((128, 2), np.float32)
    r4 = (r ** Q).astype(np.float64)
    rcol[:, 0] = r4[:128]
    rcol[:, 1] = r4[128:]

    return dict(bw=bw, cw=cw, m2w=m2w, epre=epre, rcol=rcol)


def _prep_xin(core_inputs_f32):
    """[BPC, L, H] f32 -> [BPC, H, Q, K] fp16 with xin[b,h,j,k] = u[b,4k+j,h]."""
    a = core_inputs_f32.reshape(BPC, K, Q, HIDDEN)
    return np.ascontiguousarray(a.transpose(0, 3, 2, 1)).astype(np.float16)


def _make_in_maps(inputs, A_diag, G_diag, steps, B, C, D):
    inputs = np.asarray(inputs, np.float32)
    consts = _host_constants(np.asarray(A_diag), np.asarray(G_diag),
                             np.asarray(steps), np.asarray(B), np.asarray(C),
                             np.asarray(D))
    in_maps = []
    for core in range(N_CORES):
        m = dict(consts)
        m["xin"] = _prep_xin(inputs[BPC * core: BPC * (core + 1)])
        in_maps.append(m)
    return in_maps


def kernel(inputs, A_diag, G_diag, steps, B, C, D):
    from concourse import bass_utils

    in_maps = _make_in_maps(inputs, A_diag, G_diag, steps, B, C, D)
    if "prog" not in _COMPILED:
        _COMPILED["prog"] = _build_program()
    nc = _COMPILED["prog"]

    res = bass_utils.run_bass_kernel_spmd(nc, in_maps,
                                          core_ids=list(range(N_CORES)))
    out = np.concatenate([res.results[i]["out"] for i in range(N_CORES)],
                         axis=0)                      # [B, Q, H, K] fp16
    # out[b, j, h, k] -> full[b, 4k+j, h]
    full = out.astype(np.float32).transpose(0, 3, 1, 2)   # [B, K, Q, H]
    return np.ascontiguousarray(full.reshape(BATCH, LENGTH, HIDDEN))


# revision 5
# speedup vs baseline: 2.6299x; 1.0685x over previous
"""DampedLinOSSLayer Trainium2 kernel v5 (8 NeuronCores, batch-sharded).

Radix-4 time decimation on top of the gauge-factorized scan:
  x_t = lam x_{t-1} + B u_t  decimated by Q=4: z_k = lam^4 z_{k-1} + d_k with
  d_k = sum_j lam^{3-j} B u_{4k+j}.  The lam^{3-j} factors fold into 4 copies
  of the B weights, so the intra-group reduction runs on the PE (4
  accumulating matmuls over phase-deinterleaved input columns).  The DVE only
  rotates/scans the decimated (L/4) grid: gauge y_k = r^4 y_{k-1} + cf_k with
  cf = e^{-i4th k} d; one merged scan instruction covers all four
  (half, comp) segments via zeroed coefficient columns at segment starts.
  Output for t = 4k+j:
    out = Re(C lam^{j+1} z_{k-1}) + sum_{i<=j} M_{j-i} u_{4k+i}
  with M_d = Re(C lam^d B) + diag(D)[d=0].  The z-planes are combined on the
  DVE (z_re, z_im) so the C projection is 2 matmuls per (half, phase); the
  one-column shift of z rides the matmul output AP.  The u-term is a
  10-matmul causal phase convolution.  All rotation tables are plane-views
  of a single e^{+-i4th k} table.

Host side: input is pre-transposed/deinterleaved to [H, 4, 512] per batch
(plain contiguous DMA, no device transpose); output phases [4, H, 512] fp16
are re-interleaved on the host.
"""

import numpy as np

BATCH, LENGTH, HIDDEN, P = 32, 2048, 128, 256
N_CORES = 8
BPC = BATCH // N_CORES
Q = 4
K = LENGTH // Q

XIN_DTYPE = np.float16

_COMPILED = {}


def _build_program():
    import concourse.bacc as bacc
    import concourse.mybir as mybir
    from concourse.tile import TileContext

    f32 = mybir.dt.float32
    fp16 = mybir.dt.float16
    mm = mybir.AluOpType.mult
    ad = mybir.AluOpType.add

    nc = bacc.Bacc("TRN2", target_bir_lowering=False, debug=False,
                   num_devices=N_CORES)

    # host-deinterleaved input: xin[b, h, j, k] = u[b, 4k+j, h]
    xin = nc.dram_tensor("xin", [BPC, HIDDEN, Q, K], fp16,
                         kind="ExternalInput").ap()
    # B weights with lam^{3-j} folded: [h, half, comp, j, p]
    bw = nc.dram_tensor("bw", [HIDDEN, 2, 2, Q, 128], fp16,
                        kind="ExternalInput").ap()
    # per-phase C weights C*lam^{j+1}: [p, half, j, wt(0=re,1=-im), h]
    cw = nc.dram_tensor("cw", [128, 2, Q, 2, HIDDEN], fp16,
                        kind="ExternalInput").ap()
    # phase-convolution weights M_d^T: [h_in, d, h_out]
    m2w = nc.dram_tensor("m2w", [HIDDEN, Q, HIDDEN], fp16,
                         kind="ExternalInput").ap()
    # rotation tables, phase 4*th*k: [p, half, tab, plane, k]
    #   tab0 = [cos | -sin], tab1 = [sin | cos]
    epre = nc.dram_tensor("epre", [128, 2, 2, 2, K], fp16,
                          kind="ExternalInput").ap()
    # r^4 per (p, half) for the scan coefficient
    rcol = nc.dram_tensor("rcol", [128, 2], f32, kind="ExternalInput").ap()
    # output phases; host interleaves
    out = nc.dram_tensor("out", [BPC, Q, HIDDEN, K], fp16,
                         kind="ExternalOutput").ap()

    with TileContext(nc) as tc:
        with (
            tc.tile_pool(name="const", bufs=1) as cpool,
            tc.tile_pool(name="intp", bufs=3) as intr_pool,
            tc.tile_pool(name="dsb", bufs=2) as dsb_pool,
            tc.tile_pool(name="t12", bufs=2) as t12_pool,
            tc.tile_pool(name="cbuf", bufs=2) as cbuf_pool,
            tc.tile_pool(name="ybuf", bufs=2) as ybuf_pool,
            tc.tile_pool(name="xbuf", bufs=2) as xbuf_pool,
            tc.tile_pool(name="obuf", bufs=8) as obuf_pool,
            tc.tile_pool(name="psb", bufs=2, space="PSUM") as psb,
            tc.tile_pool(name="pso", bufs=4, space="PSUM") as pso,
        ):
            bw_t = cpool.tile([HIDDEN, 2, 2, Q, 128], fp16, tag="bw")
            cw_t = cpool.tile([128, 2, Q, 2, HIDDEN], fp16, tag="cw")
            m2w_t = cpool.tile([HIDDEN, Q, HIDDEN], fp16, tag="m2w")
            epre_t = cpool.tile([128, 2, 2, 2, K], fp16, tag="epre")
            rcol_t = cpool.tile([128, 2], f32, tag="rcol")
            # DMA order = need order: batch-0 B-proj wants bw+xin first, the
            # first pre-rotation wants epre+rcol, the C projection cw/m2w.
            inT = [None] * (BPC + 1)
            inT[0] = intr_pool.tile([HIDDEN, Q, K], fp16, tag="inT",
                                    name="inT0")
            for srcap, dst in [(rcol, rcol_t[:]), (bw[:, 0], bw_t[:, 0]),
                               (xin[0], inT[0][:]), (bw[:, 1], bw_t[:, 1]),
                               (epre[:, 0], epre_t[:, 0]),
                               (epre[:, 1], epre_t[:, 1]),
                               (m2w, m2w_t[:]), (cw, cw_t[:])]:
                nc.sync.dma_start(dst, srcap[:] if hasattr(srcap, 'ap') else srcap)

            # scan coefficient [128, (half comp k)]: r^4, zero at segment
            # starts so one scan instruction covers 4 independent segments.
            rz_t = cpool.tile([128, 2, 2, K], f32, tag="rz")
            for half in range(2):
                nc.vector.memset(rz_t[:, half], 1.0)
                nc.vector.tensor_scalar_mul(
                    rz_t[:, half], rz_t[:, half],
                    rcol_t[:, half:half + 1])
            nc.vector.memset(rz_t[:, :, :, 0:1], 0.0)

            dsb = [None] * (BPC + 1)

            def bproj(b):
                # B-projection of batch b: 4 accumulating matmuls per
                # (half, comp) over the phase-deinterleaved input columns,
                # then PSUM->SBUF fp16 copies into one [p, half, comp, k]
                # tile.
                ds = dsb_pool.tile([128, 2, 2, K], fp16, tag="ds",
                                   name=f"ds{b}")
                for half in range(2):
                    d = psb.tile([128, 2, K], f32, tag="d")
                    for comp in range(2):
                        for j in range(Q):
                            nc.tensor.matmul(
                                d[:, comp], bw_t[:, half, comp, j],
                                inT[b][:, j], start=(j == 0), stop=(j == Q - 1))
                    nc.scalar.copy(ds[:, half], d[:])
                dsb[b] = ds

            bproj(0)
            for b in range(BPC):
                # prefetch + B-proj one batch ahead so the DVE chain of
                # batch b never waits on the PE/ScalarE front end.
                if b + 1 < BPC:
                    inT[b + 1] = intr_pool.tile([HIDDEN, Q, K], fp16,
                                                tag="inT",
                                                name=f"inT{b + 1}")
                    nc.sync.dma_start(inT[b + 1][:], xin[b + 1])
                    bproj(b + 1)

                # ---- per-half DVE chain; h0 scan/post overlap h1 ----
                cf = cbuf_pool.tile([128, 2, 2, K], fp16, tag="cf",
                                    name=f"cf{b}")
                yb = ybuf_pool.tile([128, 2, 2, K], fp16, tag="y",
                                    name=f"y{b}")
                zt = xbuf_pool.tile([128, 2, 2, K], fp16, tag="zt",
                                    name=f"zt{b}")

                def pre_rot(half):
                    # cf_re = d_re cos + d_im sin ; cf_im = -d_re sin + d_im cos
                    t12 = t12_pool.tile([128, 2, 2, K], fp16, tag="t12",
                                        name=f"t12_{b}_{half}")
                    nc.vector.tensor_mul(
                        t12[:, 0],
                        dsb[b][:, half, 0:1, :].to_broadcast([128, 2, K]),
                        epre_t[:, half, 0])
                    nc.vector.tensor_mul(
                        t12[:, 1],
                        dsb[b][:, half, 1:2, :].to_broadcast([128, 2, K]),
                        epre_t[:, half, 1])
                    nc.vector.tensor_add(cf[:, half], t12[:, 0], t12[:, 1])

                def scan(half):
                    nc.vector.tensor_tensor_scan(
                        yb[:, half].rearrange("p c k -> p (c k)"),
                        rz_t[:, half].rearrange("p c k -> p (c k)"),
                        cf[:, half].rearrange("p c k -> p (c k)"), 0.0,
                        op0=mm, op1=ad)

                def post_rot(half):
                    # z_re = y_re cos - y_im sin ; z_im = y_re sin + y_im cos
                    t34 = t12_pool.tile([128, 2, 2, K], fp16, tag="t12",
                                        name=f"t34_{b}_{half}")
                    nc.vector.tensor_mul(
                        t34[:, 0],
                        yb[:, half, 0:1, :].to_broadcast([128, 2, K]),
                        epre_t[:, half, :, 0, :])
                    nc.vector.tensor_mul(
                        t34[:, 1],
                        yb[:, half, 1:2, :].to_broadcast([128, 2, K]),
                        epre_t[:, half, :, 1, :])
                    nc.vector.tensor_add(zt[:, half], t34[:, 0], t34[:, 1])

                pre_rot(0)
                scan(0)
                pre_rot(1)
                post_rot(0)
                scan(1)
                post_rot(1)
                dsb[b] = None

                # ---- u-phase convolution first (no DVE dependency) ----
                oj = [None] * Q
                for j in range(Q):
                    oj[j] = pso.tile([HIDDEN, K], f32, tag="oj",
                                     name=f"oj{b}_{j}")
                    for i in range(j + 1):
                        nc.tensor.matmul(oj[j][:], m2w_t[:, j - i],
                                         inT[b][:, i],
                                         start=(i == 0), stop=False)
                # ---- z-term, half-major so h0 runs during h1's scan ----
                for half in range(2):
                    for j in range(Q):
                        for wt in range(2):
                            last = (half == 1 and wt == 1)
                            nc.tensor.matmul(
                                oj[j][:, 1:K], cw_t[:, half, j, wt],
                                zt[:, half, wt, 0:K - 1],
                                start=False, stop=last)
                for j in range(Q):
                    oT = obuf_pool.tile([HIDDEN, K], fp16, tag="oT")
                    nc.scalar.copy(oT[:], oj[j][:])
                    nc.sync.dma_start(out[b, j], oT[:])

    nc.compile()
    return nc


def _host_constants(A_diag, G_diag, steps, B, C, D):
    A = A_diag.astype(np.float64)
    G = G_diag.astype(np.float64)
    st = steps.astype(np.float64)
    step = 1.0 / (1.0 + np.exp(-st))
    g = np.maximum(G, 0.0)
    denom = np.maximum(step * step, 1e-6)
    s = step * g
    base = np.sqrt(np.maximum(1.0 + s, 1e-6))
    a_low = (2.0 + s - 2.0 * base) / denom
    a_high = (2.0 + s + 2.0 * base) / denom
    a = a_low + np.maximum(A - a_low, 0.0) - np.maximum(A - a_high, 0.0)
    S = 1.0 / (1.0 + step * g)
    T = S + 1.0 - step * step * S * a
    imag = np.sqrt(np.maximum(S - 0.25 * T * T, 0.0))
    lam = 0.5 * T + 1j * imag
    r = np.abs(lam)
    th = np.angle(lam)

    Bc = B[..., 0].astype(np.float64) + 1j * B[..., 1].astype(np.float64)
    Cc = C[..., 0].astype(np.float64) + 1j * C[..., 1].astype(np.float64)

    # bw[h, half, comp, j, p] = {Re,Im}(lam^{3-j} Bc)[p, h]
    bw = np.zeros((HIDDEN, 2, 2, Q, 128), np.float16)
    # cw[p, half, j, wt, h]: wt0 = Re(C lam^{j+1})^T, wt1 = -Im(C lam^{j+1})^T
    cw = np.zeros((128, 2, Q, 2, HIDDEN), np.float16)
    for j in range(Q):
        Wj = (lam ** (Q - 1 - j))[:, None] * Bc          # [P, H]
        Cj = Cc * (lam ** (j + 1))[None, :]              # [H, P]
        for half in range(2):
            psl = slice(128 * half, 128 * (half + 1))
            bw[:, half, 0, j] = Wj.real[psl].T
            bw[:, half, 1, j] = Wj.imag[psl].T
            cw[:, half, j, 0] = Cj.real[:, psl].T
            cw[:, half, j, 1] = -Cj.imag[:, psl].T

    # m2w[h_in, d, h_out] = M_d^T with M_d = Re(C lam^d B) (+diag(D) at d=0)
    m2w = np.zeros((HIDDEN, Q, HIDDEN), np.float16)
    for d in range(Q):
        Md = np.real(Cc @ ((lam ** d)[:, None] * Bc))
        if d == 0:
            Md = Md + np.diag(D.astype(np.float64))
        m2w[:, d] = Md.T

    # rotation tables, phase phi = 4*th*k: tab0 = [cos|-sin], tab1 = [sin|cos]
    kk = np.arange(K, dtype=np.float64)
    cos_m = np.cos(Q * th[:, None] * kk[None, :])
    sin_m = np.sin(Q * th[:, None] * kk[None, :])
    epre = np.zeros((128, 2, 2, 2, K), np.float16)
    for half in range(2):
        psl = slice(128 * half, 128 * (half + 1))
        epre[:, half, 0, 0] = cos_m[psl]
        epre[:, half, 0, 1] = -sin_m[psl]
        epre[:, half, 1, 0] = sin_m[psl]
        epre[:, half, 1, 1] = cos_m[psl]

    rcol = np.zeros((128, 2), np.float32)
    r4 = (r ** Q).astype(np.float64)
    rcol[:, 0] = r4[:128]
    rcol[:, 1] = r4[128:]

    return dict(bw=bw, cw=cw, m2w=m2w, epre=epre, rcol=rcol)


def _prep_xin(core_inputs_f32):
    """[BPC, L, H] f32 -> [BPC, H, Q, K] fp16 with xin[b,h,j,k] = u[b,4k+j,h]."""
    a = core_inputs_f32.reshape(BPC, K, Q, HIDDEN)
    return np.ascontiguousarray(a.transpose(0, 3, 2, 1)).astype(np.float16)


def _make_in_maps(inputs, A_diag, G_diag, steps, B, C, D):
    inputs = np.asarray(inputs, np.float32)
    consts = _host_constants(np.asarray(A_diag), np.asarray(G_diag),
                             np.asarray(steps), np.asarray(B), np.asarray(C),
                             np.asarray(D))
    in_maps = []
    for core in range(N_CORES):
        m = dict(consts)
        m["xin"] = _prep_xin(inputs[BPC * core: BPC * (core + 1)])
        in_maps.append(m)
    return in_maps


def kernel(inputs, A_diag, G_diag, steps, B, C, D):
    from concourse import bass_utils

    in_maps = _make_in_maps(inputs, A_diag, G_diag, steps, B, C, D)
    if "prog" not in _COMPILED:
        _COMPILED["prog"] = _build_program()
    nc = _COMPILED["prog"]

    res = bass_utils.run_bass_kernel_spmd(nc, in_maps,
                                          core_ids=list(range(N_CORES)))
    out = np.concatenate([res.results[i]["out"] for i in range(N_CORES)],
                         axis=0)                      # [B, Q, H, K] fp16
    # out[b, j, h, k] -> full[b, 4k+j, h]
    full = out.astype(np.float32).transpose(0, 3, 1, 2)   # [B, K, Q, H]
    return np.ascontiguousarray(full.reshape(BATCH, LENGTH, HIDDEN))
